# revision 15
# baseline (speedup 1.0000x reference)
"""Causal single-head attention on 8 Trainium2 NeuronCores (Bass/Tile).

Problem: X [4, 2048, 1024] f32; W_q/W_k/W_v [1024, 1024] f32.
out[b] = softmax(mask((X[b] Wq)(X[b] Wk)^T / 32)) (X[b] Wv)

Sharding: 8 cores = 4 batches x 2 key-parity halves (partial softmax).
Core c = 2b + h owns batch b's key tiles {2j + h : j = 0..7} (128-row
tiles, interleaved so causal work per local tile j is j-independent across
cores). Each core computes unnormalized partial attention over its own
keys only and returns the partial numerator [2048, 1024] plus partial
softmax denominators; the host adds each pair's partials and divides.
Since exp needs no max-subtraction here (|scores/32| < ~4), partial
softmax combines exactly.

Merged QK weight: scores = (X Wq)(Xk Wk)^T = X (Wq Wk^T) Xk^T, so the
host folds W := Wk Wq^T once and the kernel computes scores as
(Xk W) X^T - the Q projection becomes a raw DMA of X^T.

q-block permutation: the SPMD program is shared by both parities, so the
host packs each core's X^T with 128-col q-blocks permuted so that the
core's OWN key blocks always sit at even block positions (h=1 swaps each
adjacent block pair). The kernel then reads its own-key hi columns
directly out of the q tiles (no separate xk-hi DMA), the causal band
mask is per-core input data, and the host un-permutes the output rows.

fp8 DoubleRow matmuls: the PE runs fp8e4/e5 matmuls with
perf_mode=DoubleRow (two 128-row K-blocks per instruction) at 4x the
bf16 column rate. Precision is recovered where needed with a split
representation a = a_h + a_l (hi: e4m3, residual: e5m2). Mix used here
(~1.97e-2 < 2e-2 gate, verified in numpy emulation + on hardware):
  KW-proj  = wkq pure e4m3 (x64 host scale)  (x)  xk hi        [1-term]
  scores   = KW pure e4m3 (psum copy)        (x)  xq hi        [1-term]
  V-proj   = xk split  (x)  wv split (x32 host scale)          [3-term]
  AV       = w split (exp output)  (x)  V split                [3-term]
Power-of-2 scales (wkq x64, wv x32) keep the tiny merged weights out of
e4m3's subnormal floor; they fold into the exp scale (1/2048) and the
host's final divide. The numerator ships as fp16 (3 more mantissa bits
than bf16 at the same DMA cost).

Schedule: one merged stream. xq arrives as 8 x 256-col chunks
interleaved with the 8 wm tiles so the KW projection starts ~1.2us in
(a short Pool-paced warm-up keeps the PE p-state streak alive until
then). Scores tiles S(j) interleave with the V projection pieces
(V gated on the later wv/xk-l DMAs, S on the already-resident KW/xq),
then the 16 AV emits run back-to-back at the end - by then every w
split has long completed, so the tail is just the last copy+DMA chain.
All matmul contractions keep the contracted dim on partitions; DoubleRow
pairs adjacent 128-blocks (d-pairs for projections, e-pairs for scores,
own-key-tile jj-pairs for AV, odd counts padded with a zero block).
Attention weights live transposed (own keys on partitions) so they are
directly usable as matmul lhsT for numerator and denominator.
"""

import sys

if "/opt/trn_rl_repo" not in sys.path:
    sys.path.insert(0, "/opt/trn_rl_repo")

import numpy as np

B, S, D = 4, 2048, 1024
HK = S // 2  # own key rows per core
P = 128
N_CORES = 8
# column offset of attention-weight block j inside the packed wt tile
WOFF = [0] * 9
for _j in range(8):
    WOFF[_j + 1] = WOFF[_j] + (16 - 2 * _j) * P
WTW = WOFF[8]  # 9216; cols [WTW, WTW+128) are a permanent zero pad block

_cache = {}


def _build_nc():
    from concourse import bacc
    import concourse.mybir as mybir
    import concourse.tile as tile

    fp32 = mybir.dt.float32
    f16 = mybir.dt.float16
    bf16 = mybir.dt.bfloat16
    f8h = mybir.dt.float8e4  # e4m3: hi parts + pure operands
    f8l = mybir.dt.float8e5  # e5m2: residual parts
    Exp = mybir.ActivationFunctionType.Exp
    Copy = mybir.ActivationFunctionType.Copy
    DR = mybir.MatmulPerfMode.DoubleRow

    nc = bacc.Bacc("TRN2", target_bir_lowering=False)

    # host-packed inputs, already in SBUF tile layout (partition-major)
    wm_d = [
        nc.dram_tensor(f"wm{e}", [P, 8, P], f8h, kind="ExternalInput")
        for e in range(8)
    ]
    xq_d = [
        nc.dram_tensor(f"xq{c}", [P, 8, 256], f8h, kind="ExternalInput")
        for c in range(8)
    ]
    xk1l_d = nc.dram_tensor("xk1l", [P, 8, 512], f8l, kind="ExternalInput")
    xk2l_d = nc.dram_tensor("xk2l", [P, 8, 512], f8l, kind="ExternalInput")
    wvh0_d = nc.dram_tensor("wvh0", [P, 8, 512], f8h, kind="ExternalInput")
    wvl0_d = nc.dram_tensor("wvl0", [P, 8, 512], f8l, kind="ExternalInput")
    wvh1_d = nc.dram_tensor("wvh1", [P, 8, 512], f8h, kind="ExternalInput")
    wvl1_d = nc.dram_tensor("wvl1", [P, 8, 512], f8l, kind="ExternalInput")
    band_d = nc.dram_tensor("band", [P, 256], fp32, kind="ExternalInput")
    # partial numerator (x32) in fp16 (halves output DMA; the host pair-sum
    # and divide run in fp32) + per-q-tile denominator columns in fp32
    num_d = nc.dram_tensor("num", [S, D], f16, kind="ExternalOutput")
    den_d = nc.dram_tensor("den", [P, 16], fp32, kind="ExternalOutput")

    with tile.TileContext(nc) as tc:
        with (
            tc.tile_pool(name="persist", bufs=1) as persist,
            tc.tile_pool(name="wtp", bufs=1) as wtp,
            tc.tile_pool(name="tmpp", bufs=8) as tmpp,
            tc.tile_pool(name="outp", bufs=6) as outp,
            tc.tile_pool(name="warm", bufs=1) as warmp,
            tc.tile_pool(name="psP", bufs=4, space="PSUM") as psP,
            tc.tile_pool(name="psS", bufs=3, space="PSUM") as psS,
            tc.tile_pool(name="psD", bufs=1, space="PSUM") as psD,
        ):
            XQ = [
                persist.tile([P, 8, 256], f8h, tag=f"xq{c}", name=f"xq{c}")
                for c in range(8)
            ]
            Wm = [
                persist.tile([P, 8, P], f8h, tag=f"wm{e}", name=f"wm{e}")
                for e in range(8)
            ]
            XKL = [
                persist.tile([P, 8, 512], f8l, tag=f"xkl{sc}", name=f"xkl{sc}")
                for sc in range(2)
            ]
            WVH = persist.tile([P, 8, D], f8h, tag="wvh")
            WVL = persist.tile([P, 8, D], f8l, tag="wvl")
            KW = persist.tile([P, 8, HK], f8h, tag="kw")  # (Xk Wk Wq^T)^T x64
            VH = persist.tile([P, 8, D], f8h, tag="vh")  # Xk Wv x32 hi
            VL = persist.tile([P, 8, D], f8l, tag="vl")  # .. residual
            band = persist.tile([P, 256], fp32, tag="band")
            onesH = persist.tile([P, 2, 1], f8h, tag="onesH")
            onesL = persist.tile([P, 2, 1], f8l, tag="onesL")
            den_sb = persist.tile([P, 16], fp32, tag="den")  # col per q-tile
            dummy = persist.tile([P, 1], fp32, tag="dummy")
            nc.vector.memset(onesH[:], 1.0)
            nc.vector.memset(onesL[:], 1.0)
            # preload the Act engine's Exp table during the DMA-bound start
            nc.scalar.activation(dummy[:], onesH[:, 0], Exp, scale=1.0)

            # PE warm-up: matmul cost is halved only once the PE's busy
            # streak is ~3us old, and the streak survives sub-~0.8us idle
            # gaps. The first input DMAs take ~1.2us, so keep the streak
            # alive with tiny N=64 matmuls paced ~450ns apart by a Pool
            # copy-chain (ping-pong buffers force serialization; Pool
            # starts instantly and is otherwise idle at the start).
            wl_ = warmp.tile([P, P], bf16, tag="warm_l")
            wa = warmp.tile([P, 256], bf16, tag="warm_a")
            wb = warmp.tile([P, 256], bf16, tag="warm_b")
            nc.gpsimd.memset(wl_[:], 0.0)
            nc.gpsimd.memset(wb[:], 0.0)
            ps_w = psP.tile([P, 512], fp32, tag="psP", name="warm")
            for i in range(3):
                src, dst = (wb, wa) if i % 2 == 0 else (wa, wb)
                nc.gpsimd.tensor_copy(dst[:], src[:])
                nc.tensor.matmul(
                    ps_w[:, :64], wl_[:], dst[:, :64], start=True, stop=True
                )

            # DMA issue order = arrival order: wm/xq interleaved first (KW
            # needs both), then band, then the V-projection operands.
            nc.sync.dma_start(Wm[0][:], wm_d[0][:])
            nc.sync.dma_start(XQ[0][:], xq_d[0][:])
            nc.sync.dma_start(Wm[1][:], wm_d[1][:])
            nc.sync.dma_start(XQ[1][:], xq_d[1][:])
            nc.sync.dma_start(Wm[2][:], wm_d[2][:])
            nc.sync.dma_start(XQ[2][:], xq_d[2][:])
            nc.sync.dma_start(Wm[3][:], wm_d[3][:])
            nc.sync.dma_start(XQ[3][:], xq_d[3][:])
            nc.sync.dma_start(Wm[4][:], wm_d[4][:])
            nc.sync.dma_start(Wm[5][:], wm_d[5][:])
            nc.sync.dma_start(Wm[6][:], wm_d[6][:])
            nc.sync.dma_start(Wm[7][:], wm_d[7][:])
            nc.sync.dma_start(XQ[4][:], xq_d[4][:])
            nc.sync.dma_start(XQ[5][:], xq_d[5][:])
            nc.sync.dma_start(XQ[6][:], xq_d[6][:])
            nc.sync.dma_start(XQ[7][:], xq_d[7][:])
            nc.sync.dma_start(band[:], band_d[:])
            nc.sync.dma_start(WVH[:, :, 0:512], wvh0_d[:])
            nc.sync.dma_start(WVL[:, :, 0:512], wvl0_d[:])
            nc.sync.dma_start(XKL[0][:], xk1l_d[:])
            nc.sync.dma_start(WVH[:, :, 512:1024], wvh1_d[:])
            nc.sync.dma_start(WVL[:, :, 512:1024], wvl1_d[:])
            nc.sync.dma_start(XKL[1][:], xk2l_d[:])

            # ---- KW projection: KW[e, own keys] = (x64 merged W)^T @ Xk^T.
            # 1-term fp8 DR, contraction d=1024 as 4 dp-pairs. The own-key
            # hi columns live in the first 128 cols of each q chunk tile, so
            # each 512-key psum fills as 4 independent 128-col groups gated
            # on individual chunk arrivals.
            def kw_proj(sc):
                for e in range(8):
                    psum = psP.tile([P, 512], fp32, tag="psP")
                    for jj in range(4):
                        xc = XQ[4 * sc + jj]
                        for dp in range(4):
                            nc.tensor.matmul(
                                psum[:, jj * P : (jj + 1) * P],
                                Wm[e][:, 2 * dp : 2 * dp + 2, :],
                                xc[:, 2 * dp : 2 * dp + 2, 0:P],
                                start=(dp == 0),
                                stop=(dp == 3),
                                perf_mode=DR,
                            )
                    # evacuation split Act+Act/DVE halves: a single 512-col
                    # copy (>600ns) is slower than the 427ns psum fill, so
                    # full-width evacs stall the 4-deep psum rotation
                    nc.scalar.activation(
                        KW[:, e, sc * 512 : sc * 512 + 256], psum[:, :256], Copy
                    )
                    nc.vector.tensor_copy(
                        KW[:, e, sc * 512 + 256 : (sc + 1) * 512], psum[:, 256:]
                    )

            # ---- V projection piece: V[own tile 4sc+kti, e-half ec].
            # 3-term split x split (drop l*l); the xk-l term last so the
            # late xk-l DMA gates as little as possible.
            def v_proj(sc, ec):
                for kti in range(4):
                    kt = 4 * sc + kti
                    psum = psP.tile([P, 512], fp32, tag="psP")
                    for t in range(3):
                        for dp in range(4):
                            if t < 2:
                                lhs = XQ[kt][:, 2 * dp : 2 * dp + 2, 0:P]
                                wv = (WVH, WVL)[t]
                            else:
                                lhs = XKL[sc][
                                    :, 2 * dp : 2 * dp + 2, kti * P : (kti + 1) * P
                                ]
                                wv = WVH
                            nc.tensor.matmul(
                                psum[:],
                                lhs,
                                wv[:, 2 * dp : 2 * dp + 2, ec * 512 : (ec + 1) * 512],
                                start=(t == 0 and dp == 0),
                                stop=(t == 2 and dp == 3),
                                perf_mode=DR,
                            )
                    nc.scalar.activation(
                        VH[:, kt, ec * 512 : (ec + 1) * 512], psum[:], Copy
                    )
                    nc.vector.tensor_sub(
                        VL[:, kt, ec * 512 : (ec + 1) * 512],
                        psum[:],
                        VH[:, kt, ec * 512 : (ec + 1) * 512],
                    )

            # ---- scores + exp + w split for own key tile j, q in
            # [256j, 2048), processed as 512-col psums (two 256-col chunk
            # groups each) to halve per-op overhead on Act/Pool/DVE.
            wtH = wtp.tile([P, WTW + P], f8h, tag="wth")
            wtL = wtp.tile([P, WTW + P], f8l, tag="wtl")
            nc.vector.memset(wtH[:, WTW:], 0.0)  # zero pad block for
            nc.vector.memset(wtL[:, WTW:], 0.0)  # odd jj-pair counts
            # 128-col-block views for strided jj-pair lhsT access
            wtHv = wtH[:].rearrange("p (n b) -> p n b", b=P)
            wtLv = wtL[:].rearrange("p (n b) -> p n b", b=P)

            def scores_for(j):
                nch = 8 - j
                ch = 0
                while ch < nch:
                    wide = 512 if ch + 1 < nch else 256
                    psum_s = psS.tile([P, 512], fp32, tag="psS")
                    for sub in range(wide // 256):
                        xc = XQ[j + ch + sub]
                        for ep in range(4):
                            nc.tensor.matmul(
                                psum_s[:, sub * 256 : (sub + 1) * 256],
                                KW[:, 2 * ep : 2 * ep + 2, j * P : (j + 1) * P],
                                xc[:, 2 * ep : 2 * ep + 2, :],
                                start=(ep == 0),
                                stop=(ep == 3),
                                perf_mode=DR,
                            )
                    # psum holds 2048*z (64 from wkq, 32 softmax scale)
                    tmp = tmpp.tile([P, 512], fp32, tag="tmp")
                    nc.scalar.activation(
                        tmp[:, :wide], psum_s[:, :wide], Exp, scale=1 / 2048.0
                    )
                    if ch == 0:
                        # diagonal block: causal 0/1 mask (parity in data)
                        nc.gpsimd.tensor_mul(tmp[:, :256], tmp[:, :256], band[:])
                    wcol = WOFF[j] + 256 * ch
                    nc.gpsimd.tensor_copy(wtH[:, wcol : wcol + wide], tmp[:, :wide])
                    nc.vector.tensor_sub(
                        wtL[:, wcol : wcol + wide],
                        tmp[:, :wide],
                        wtH[:, wcol : wcol + wide],
                    )
                    ch += wide // 256

            # ---- AV emit for q tile g: denominator + numerator via the
            # same DoubleRow jj-pairs (zero pad block absorbs odd counts).
            def emit(g):
                nj = g // 2 + 1  # own key tiles jj with 2jj <= g
                npair = (nj + 1) // 2
                out_sb = outp.tile([P, D], f16, tag="out")

                def pair_ap(wv_, pp):
                    # lhsT [P, 2, P]: w blocks jj=2pp, 2pp+1 for this
                    # g; an odd tail pairs with the zero pad block.
                    jj = 2 * pp
                    c0 = WOFF[jj] // P + (g - 2 * jj)
                    if jj + 1 < nj:
                        c1 = WOFF[jj + 1] // P + (g - 2 * jj - 2)
                    else:
                        c1 = WTW // P
                    step = c1 - c0
                    return wv_[:, c0 :: step, :][:, 0:2, :]

                psum_dn = psD.tile([P, 1], fp32, tag="psD")
                for t, (wv_, on) in enumerate(((wtHv, onesH), (wtLv, onesL))):
                    for pp in range(npair):
                        nc.tensor.matmul(
                            psum_dn[:],
                            pair_ap(wv_, pp),
                            on[:],
                            start=(t == 0 and pp == 0),
                            stop=(t == 1 and pp == npair - 1),
                            perf_mode=DR,
                        )
                nc.scalar.activation(den_sb[:, g : g + 1], psum_dn[:], Copy)
                if g == 15:
                    # batched denominator: one tiny contiguous DMA,
                    # issued before the final AV so it is off the tail
                    nc.sync.dma_start(den_d[:], den_sb[:])
                for ec in range(2):
                    psum_av = psP.tile([P, 512], fp32, tag="psP")
                    terms = ((wtHv, VH), (wtLv, VH), (wtHv, VL))
                    for t, (wv_, vv) in enumerate(terms):
                        for pp in range(npair):
                            nc.tensor.matmul(
                                psum_av[:],
                                pair_ap(wv_, pp),
                                vv[:, 2 * pp : 2 * pp + 2, ec * 512 : (ec + 1) * 512],
                                start=(t == 0 and pp == 0),
                                stop=(t == 2 and pp == npair - 1),
                                perf_mode=DR,
                            )
                    # e-half DMA right after its copy: the final
                    # copy->DMA chains overlap instead of serializing
                    if ec == 0:
                        nc.scalar.activation(out_sb[:, :512], psum_av[:], Copy)
                        nc.sync.dma_start(
                            num_d[g * P : (g + 1) * P, :512], out_sb[:, :512]
                        )
                    elif g == 15:
                        # final emit: split the last evacuation across DVE
                        # and Act (parallel 256-col copies), both DMAs on
                        # the free SP ring, so the tail chain is one short
                        # copy plus pipelined DMA issues
                        nc.vector.tensor_copy(out_sb[:, 512:768], psum_av[:, :256])
                        nc.scalar.activation(
                            out_sb[:, 768:1024], psum_av[:, 256:], Copy
                        )
                        nc.sync.dma_start(
                            num_d[g * P : (g + 1) * P, 512:768], out_sb[:, 512:768]
                        )
                        nc.sync.dma_start(
                            num_d[g * P : (g + 1) * P, 768:], out_sb[:, 768:]
                        )
                    else:
                        nc.vector.tensor_copy(out_sb[:, 512:1024], psum_av[:])
                if g != 15:
                    nc.sync.dma_start(
                        num_d[g * P : (g + 1) * P, 512:], out_sb[:, 512:]
                    )

            # ---- the merged stream
            kw_proj(0)
            scores_for(0)
            kw_proj(1)
            scores_for(1)
            v_proj(0, 0)
            scores_for(2)
            v_proj(0, 1)
            scores_for(3)
            v_proj(1, 0)
            scores_for(4)
            v_proj(1, 1)
            scores_for(5)
            emit(0)
            emit(1)
            scores_for(6)
            emit(2)
            emit(3)
            scores_for(7)
            for g in range(4, 16):
                emit(g)

    nc.compile()
    return nc


def _get_nc():
    if "nc" not in _cache:
        _cache["nc"] = _build_nc()
    return _cache["nc"]


def _parity_cols(h):
    return np.concatenate(
        [np.arange(P * (2 * j + h), P * (2 * j + h) + P) for j in range(8)]
    )


def _perm_cols(h):
    """q-column order: own-key 128-blocks first within each 256 pair."""
    blocks = []
    for m in range(8):
        if h == 0:
            blocks += [2 * m, 2 * m + 1]
        else:
            blocks += [2 * m + 1, 2 * m]
    return np.concatenate([np.arange(P * b, P * b + P) for b in blocks]), blocks


def _split8(a):
    """hi (e4m3) + residual (e5m2) split of a float32 array."""
    import ml_dtypes

    a = np.asarray(a, dtype=np.float32)
    h = a.astype(ml_dtypes.float8_e4m3)
    l = (a - h.astype(np.float32)).astype(ml_dtypes.float8_e5m2)
    return h, l


def _pack(a):
    """[D, n] row-major -> [P, 8, n] partition-major tile layout."""
    n = a.shape[1]
    return np.ascontiguousarray(a.reshape(8, P, n).transpose(1, 0, 2))


def kernel(X, W_q, W_k, W_v, _run_kwargs=None, _results_out=None):
    import ml_dtypes
    from concourse.bass_utils import run_bass_kernel_spmd

    f8 = ml_dtypes.float8_e4m3
    X = np.asarray(X, dtype=np.float32)
    W_q = np.asarray(W_q, dtype=np.float32)
    W_k = np.asarray(W_k, dtype=np.float32)
    # scores = (X Wq)(Xk Wk)^T = X (Wq Wk^T) Xk^T: fold the weight product.
    # x64 scale keeps the tiny merged weights out of e4m3 subnormals; it is
    # divided back out in the exp scale (1/2048).
    wm = _pack((64.0 * (W_k @ W_q.T)).astype(f8))
    # x32 on Wv likewise; divided back out in the host's final division.
    wvh, wvl = _split8(32.0 * np.asarray(W_v, dtype=np.float32))
    wvh, wvl = _pack(wvh), _pack(wvl)

    kcols = [_parity_cols(0), _parity_cols(1)]
    qcols = [_perm_cols(0), _perm_cols(1)]
    per_batch = []
    for b in range(B):
        xqT = np.ascontiguousarray(X[b].T)
        xqh, xql = _split8(xqT)
        xqh = xqh.astype(np.float32).astype(f8)  # no-op, keeps dtype f8
        pb = []
        for h in range(2):
            xkl = np.ascontiguousarray(xql[:, kcols[h]])
            xqp = np.ascontiguousarray(xqh[:, qcols[h][0]])
            pb.append((xqp, xkl))
        per_batch.append(pb)
    # causal band for the diagonal 256-col chunk, per parity (permuted q):
    # h=0: q'=x is global 256j+x vs keys 256j+p -> x >= p
    # h=1: x<128 is global 256j+128+x vs keys 256j+128+p -> x >= p;
    #      x>=128 is global 256j+(x-128) < keys -> 0
    x = np.arange(256)[None, :]
    p = np.arange(P)[:, None]
    bands = [
        np.ascontiguousarray((x >= p).astype(np.float32)),
        np.ascontiguousarray(((x >= p) & (x < P)).astype(np.float32)),
    ]

    in_maps = []
    for c in range(N_CORES):
        b, h = divmod(c, 2)
        xqp, xkl = per_batch[b][h]
        xklp = _pack(xkl)
        in_maps.append(
            {
                **{
                    f"xq{cc}": _pack(xqp[:, cc * 256 : (cc + 1) * 256])
                    for cc in range(8)
                },
                "xk1l": np.ascontiguousarray(xklp[:, :, 0:512]),
                "xk2l": np.ascontiguousarray(xklp[:, :, 512:1024]),
                **{
                    f"wm{e}": np.ascontiguousarray(wm[:, :, e * P : (e + 1) * P])
                    for e in range(8)
                },
                "wvh0": np.ascontiguousarray(wvh[:, :, 0:512]),
                "wvh1": np.ascontiguousarray(wvh[:, :, 512:1024]),
                "wvl0": np.ascontiguousarray(wvl[:, :, 0:512]),
                "wvl1": np.ascontiguousarray(wvl[:, :, 512:1024]),
                "band": bands[h],
            }
        )

    nc = _get_nc()
    res = None
    for attempt in range(3):
        try:
            res = run_bass_kernel_spmd(
                nc, in_maps, core_ids=list(range(N_CORES)), **(_run_kwargs or {})
            )
            # materialize now: device failures surface lazily at fetch time,
            # and they must land inside this retry loop
            res.results = [
                {k: np.asarray(v) for k, v in r.items()} for r in res.results
            ]
            # a wedged exec unit can also return corrupted buffers without
            # raising - validate before accepting (den > 0 always holds:
            # every row's causal window includes at least one weight)
            for r in res.results:
                num16 = r["num"].astype(np.float32)
                if not (
                    np.all(np.isfinite(num16))
                    and np.all(np.isfinite(r["den"]))
                    and np.all(np.abs(num16) < 1e7)
                ):
                    raise RuntimeError("non-finite device output")
            break
        except Exception:
            # transient device wedges (NRT_EXEC_UNIT_UNRECOVERABLE) usually
            # clear on retry; drop the poisoned PJRT client first
            if attempt == 2:
                raise
            print(
                f"kernel: device run failed (attempt {attempt}), retrying",
                file=sys.stderr,
            )
            import time

            try:
                import jax
                import jax.extend.backend

                jax.clear_caches()
                jax.extend.backend.clear_backends()
            except Exception:
                pass
            time.sleep(3)
    if _results_out is not None:
        _results_out.append(res)

    out = np.empty((B, S, D), dtype=np.float32)
    perm1 = np.array(qcols[1][1])  # involution: permuted block -> global
    for b in range(B):
        r0, r1 = res.results[2 * b], res.results[2 * b + 1]
        num0 = r0["num"].astype(np.float32)
        # h=1 outputs are in permuted q order: un-permute the row blocks
        num1 = r1["num"].astype(np.float32).reshape(16, P, D)[perm1].reshape(S, D)
        den0 = r0["den"]
        den1 = r1["den"][:, perm1]
        num = num0 + num1
        den = (den0 + den1).T.reshape(S, 1)  # [p, g] -> row g*128+p
        # numerator carries the x32 Wv host scale; denominator does not
        out[b] = num / (32.0 * den)
    return out
